# revision 23
# baseline (speedup 1.0000x reference)
"""Causal self-attention (B=4, T=2048, D=1024, H=16) on 8 TRN2 NeuronCores.

Sharding: core i = (batch b = i//2, head-group g = i%2). Data parallel on B,
tensor parallel on heads (8 heads per group): qkv_proj columns and out_proj
rows split per head group. Each core computes a partial [D, T] output^T for
its batch; host sums the two group partials per batch, transposes, adds bias.

v2 vs baseline (396.8us): all matmuls in bf16 (1 cycle/row vs the ~2 c/r the
fp32r path measured on HW), x pre-transposed on the host (kills 128 PE
transposes + copies), everything SBUF-resident (no DRAM bounce of Q/K), the
1/sqrt(dh) scale folded into exp's scale immediate, and one unified emission
schedule: projection / output-projection units are interleaved as PE filler
between attention items so the PE never idles while the Scalar engine (exp,
~163us total, dtype-independent 1 elem/cycle/lane) grinds through softmax.

Per-core pipeline:
  proj units: Q^T/K^T[128(2 heads*64d), t] = W_pair.T @ x^T per (pair, 512-t
  chunk); V[t,d] natural per 128-t tile -> vS[k128, ktile, head, 65] with a
  ones column (softmax denominator via the AV matmul).
  attention items (2 k-tiles each): S^T[k,q] = K^T.T @ Q^T row-tiled 2 heads;
  exp(0.125*S) on ACT -> bf16 P^T; triangle mask-mul on diagonal blocks;
  AV: psum[65,512] += V'[k,d+1].T @ P^T accumulated over k-tiles (row 64 =
  denominator). Normalize with reciprocal + gpsimd partition_broadcast.
  o units: out^T[f,t] += Wo_pair[d128,f].T @ OT_pair[d128,t] over pairs.
PSUM: S pool 2x2 banks, AV 2x1, proj/outproj 2x1 = 8 banks.
"""

import numpy as np
import ml_dtypes

import concourse.bacc as bacc
import concourse.tile as tile
import concourse.mybir as mybir
from concourse import bass_utils
from concourse.bass import ts

F32 = mybir.dt.float32
BF16 = mybir.dt.bfloat16
EXP = mybir.ActivationFunctionType.Exp

T = 2048
TT = 16          # t tiles of 128
NP = 4           # head pairs per core
NQC = 4          # q chunks of 512
SCALE = 0.125    # 1/sqrt(64), folded into exp's scale immediate

_CACHE = {}
_last_in_maps = None


def _build(CT):
    """CT = number of 128-row c-tiles in the (possibly bias-augmented) x/W."""
    nc = bacc.Bacc("TRN2", target_bir_lowering=False, debug=False)
    C = CT * 128

    def mm(*args, **kwargs):
        return nc.tensor.matmul(*args, **kwargs)

    xa = nc.dram_tensor("xa", [C, T], BF16, kind="ExternalInput").ap()  # x^T
    wq = nc.dram_tensor("wq", [C, 512], BF16, kind="ExternalInput").ap()
    wk = nc.dram_tensor("wk", [C, 512], BF16, kind="ExternalInput").ap()
    wv = nc.dram_tensor("wv", [C, 512], BF16, kind="ExternalInput").ap()
    wo = nc.dram_tensor("wo", [512, 1024], BF16, kind="ExternalInput").ap()
    tri = nc.dram_tensor("tri", [128, 128], BF16, kind="ExternalInput").ap()
    ot = nc.dram_tensor("ot", [1024, T], F32, kind="ExternalOutput").ap()

    with tile.TileContext(nc) as tc:
        with (
            tc.tile_pool(name="persist", bufs=1) as persist,
            tc.tile_pool(name="ptp", bufs=6) as ptpool,
            tc.tile_pool(name="rsm", bufs=6) as rpool,
            tc.tile_pool(name="rbcp", bufs=2) as rbcpool,
            tc.tile_pool(name="obnc", bufs=3) as opool,
            tc.tile_pool(name="psS", bufs=2, space="PSUM") as psS,
            tc.tile_pool(name="psAv", bufs=2, space="PSUM") as psAv,
            tc.tile_pool(name="psP", bufs=2, space="PSUM") as psP,
        ):
            vS = persist.tile([128, TT, 8, 65], BF16)     # [k128, ktile, head, d+1]
            OT = persist.tile([128, NP, T], BF16)         # [d128(2 heads), pair, t]
            tr = persist.tile([128, 128], BF16)
            wo_sb = persist.tile([128, NP, 1024], BF16)
            qsb = persist.tile([128, NP, T], BF16)        # Q^T per pair
            ksb = persist.tile([128, NP, T], BF16)        # K^T per pair
            xt = persist.tile([128, CT, T], BF16)         # x^T tiles
            wq_sb = persist.tile([128, CT, 512], BF16)
            wk_sb = persist.tile([128, CT, 512], BF16)
            wv_sb = persist.tile([128, CT, 512], BF16)
            nc.vector.memset(vS[:, :, :, 64:65], 1.0)

            # ---- input DMAs. HBM bw (~358GB/s) is shared by every transfer
            # in flight, so the critical first-unit deps (xt chunk 0, the
            # pair-0 columns of wq/wk) are issued alone; all bulk transfers
            # are gated behind a dummy DVE read of their target regions that
            # only runs after the prologue's psum copies (see below).
            nc.sync.dma_start(
                out=xt[:, :, ts(0, 512)],
                in_=xa[:, ts(0, 512)].rearrange("(ct P) t -> P ct t", P=128),
            )
            nc.scalar.dma_start(
                out=wq_sb[:, :, 0:128],
                in_=wq[:, 0:128].rearrange("(ct P) f -> P ct f", P=128),
            )
            nc.gpsimd.dma_start(
                out=wk_sb[:, :, 0:128],
                in_=wk[:, 0:128].rearrange("(ct P) f -> P ct f", P=128),
            )
            nc.gpsimd.dma_start(
                out=wv_sb, in_=wv.rearrange("(ct P) f -> P ct f", P=128)
            )
            nc.gpsimd.dma_start(out=tr, in_=tri)

            # ---------------- PE work units ----------------
            def qk_unit(p, tc_, which):
                w_sb = wq_sb if which == 0 else wk_sb
                dst = qsb if which == 0 else ksb
                ps = psP.tile([128, 512], F32, name="pp", tag="pp")
                for cc in range(CT):
                    mm(
                        ps,
                        lhsT=w_sb[:, cc, ts(p, 128)],
                        rhs=xt[:, cc, ts(tc_, 512)],
                        start=(cc == 0),
                        stop=(cc == CT - 1),
                    )
                nc.vector.tensor_copy(out=dst[:, p, ts(tc_, 512)], in_=ps)

            def v_unit(tt):
                ps = psP.tile([128, 512], F32, name="pp", tag="pp")
                for cc in range(CT):
                    mm(
                        ps,
                        lhsT=xt[:, cc, ts(tt, 128)],
                        rhs=wv_sb[:, cc, :],
                        start=(cc == 0),
                        stop=(cc == CT - 1),
                    )
                nc.vector.tensor_copy(
                    out=vS[:, tt, :, 0:64],
                    in_=ps.rearrange("p (h d) -> p h d", h=8),
                )

            def o_unit(qc, ft):
                ps = psP.tile([128, 512], F32, name="pp", tag="pp")
                for p in range(NP):
                    mm(
                        ps,
                        lhsT=wo_sb[:, p, ts(ft, 128)],
                        rhs=OT[:, p, ts(qc, 512)],
                        start=(p == 0),
                        stop=(p == NP - 1),
                    )
                ob = opool.tile([128, 512], F32)
                nc.vector.tensor_copy(out=ob, in_=ps)
                # alternate queues; sync is idle after the input loads
                eng = nc.sync if ft % 2 == 0 else nc.gpsimd
                eng.dma_start(out=ot[ts(ft, 128), ts(qc, 512)], in_=ob)

            # ---------------- attention ----------------
            pts = {}
            avs = {}

            def s_exp(p, qc, j):
                off = max(0, 128 * j - 512 * qc)
                sg = psS.tile([128, 2, 512], F32, name="sg", tag="sg")
                jo = 512 * (j // 4) + 128 * (j % 4)
                for m in range(2):
                    mm(
                        sg[:, m, off:],
                        lhsT=ksb[64 * m : 64 * m + 64, p, jo : jo + 128],
                        rhs=qsb[64 * m : 64 * m + 64, p, 512 * qc + off : 512 * qc + 512],
                        start=True,
                        stop=True,
                    )
                ptile = ptpool.tile([128, 2, 512], BF16, name="pt", tag="pt")
                nc.scalar.activation(
                    out=ptile[:, :, off:], in_=sg[:, :, off:], func=EXP, scale=SCALE
                )
                if j >= 4 * qc:
                    nc.vector.tensor_mul(
                        ptile[:, :, off : off + 128],
                        ptile[:, :, off : off + 128],
                        tr[:, None, :].to_broadcast([128, 2, 128]),
                    )
                pts[(p, qc, j)] = (ptile, off)

            def av_mm(p, qc, j, nj):
                ptile, off = pts.pop((p, qc, j))
                av = avs[(p, qc)]
                for m in range(2):
                    mm(
                        av[m][:65, off:],
                        lhsT=vS[:, j, 2 * p + m, :],
                        rhs=ptile[:, m, off:],
                        start=(j == 0),
                        stop=(j == nj - 1),
                    )

            def normalize(p, qc):
                # fused: reciprocal straight off the PSUM denominator row,
                # broadcast, then one multiply that reads the PSUM O~ rows
                # and writes normalized bf16 OT (no intermediate copies)
                av = avs.pop((p, qc))
                rrs = []
                for m in range(2):
                    rsb = rpool.tile([1, 512], F32, name="rsb", tag="rsb")
                    nc.vector.tensor_copy(out=rsb, in_=av[m][64:65, :])
                    # unnormalized O~ out of PSUM so the av bank frees fast
                    nc.vector.tensor_copy(
                        out=OT[64 * m : 64 * m + 64, p, ts(qc, 512)],
                        in_=av[m][0:64, :],
                    )
                    rrs.append(rsb)
                for m in range(2):
                    rinv = rpool.tile([1, 512], F32, name="rinv", tag="rinv")
                    nc.vector.reciprocal_approx_fast(out=rinv, in_=rrs[m])
                    rb16 = rpool.tile([1, 512], BF16, name="rb16", tag="rb16")
                    nc.vector.tensor_copy(out=rb16, in_=rinv)
                    rb = rbcpool.tile([128, 512], BF16, name="rb", tag="rb")
                    nc.gpsimd.partition_broadcast(rb, rb16)
                    sl = OT[64 * m : 64 * m + 64, p, ts(qc, 512)]
                    nc.vector.tensor_mul(sl, sl, rb[64 * m : 64 * m + 64, :])

            def av_item(it):
                p, qc, nj, jg, first, last = it
                if first:
                    avs[(p, qc)] = [
                        psAv.tile([128, 512], F32, name="av", tag="av")
                        for _ in range(2)
                    ]
                for j in jg:
                    av_mm(p, qc, j, nj)
                if last:
                    normalize(p, qc)

            # ---------------- emission schedule ----------------
            # items: 2 k-tiles of one (pair, q-chunk); rounds by q-chunk
            rounds = []
            for qc in range(NQC):
                ritems = []
                for p in range(NP):
                    nj = 4 * qc + 4
                    js = list(range(nj))
                    sub = [js[i : i + 2] for i in range(0, nj, 2)]
                    for gi, jg in enumerate(sub):
                        ritems.append((p, qc, nj, jg, gi == 0, gi == len(sub) - 1))
                rounds.append(ritems)

            # per-round PE filler units (produce data for round r+1; drain
            # outproj of round r-1). Order within a round matters: producers
            # must precede their consumers in PE emission order, and o_units
            # of round r-1 must come after normalize(p3, r-1), which is only
            # emitted during item 1 of round r (AV lag) -> late list.
            fillers_early = [
                # round 0: r0 prereqs at the exact slots they are consumed
                # (pair p's qk before item 2p; vS tile j before its AV),
                # then round-1 projections
                [(v_unit, (0,)), (v_unit, (1,)),
                 (qk_unit, (1, 0, 0)), (qk_unit, (1, 0, 1)), (v_unit, (2,)),
                 (qk_unit, (2, 0, 0)), (qk_unit, (2, 0, 1)), (v_unit, (3,)),
                 (qk_unit, (3, 0, 0)), (qk_unit, (3, 0, 1))]
                + [(qk_unit, (p, 1, w)) for p in range(NP) for w in (0, 1)]
                + [(v_unit, (tt,)) for tt in (4, 5, 6, 7)],
                [(qk_unit, (p, 2, w)) for p in range(NP) for w in (0, 1)]
                + [(v_unit, (tt,)) for tt in (8, 9, 10, 11)],
                [(qk_unit, (p, 3, w)) for p in range(NP) for w in (0, 1)]
                + [(v_unit, (tt,)) for tt in (12, 13, 14, 15)],
                [],
            ]
            fillers_late = [
                [],
                [(o_unit, (0, ft)) for ft in range(8)],
                [(o_unit, (1, ft)) for ft in range(8)],
                [(o_unit, (2, ft)) for ft in range(8)],
            ]
            LATE_START = 4  # item index within the round where late fillers may begin

            # prologue: just enough for the first items of pair 0 (vS tiles
            # 0/1 are first fillers -- AV lags 2 items, so they land in time)
            qk_unit(0, 0, 0)
            qk_unit(0, 0, 1)

            # bulk DMAs, gated so they start only once the critical loads are
            # done: each gate op reads BOTH the bulk target region and a qsb
            # element produced by the prologue (real dependency, so the
            # scheduler cannot hoist it), and the bulk DMA write must wait
            # for that read (WAR). Later DMAs on the same queue FIFO behind.
            gate = rpool.tile([1, 8], BF16, name="gate", tag="gate")
            nc.vector.tensor_mul(gate[:, 0:1], xt[0:1, 0, 512:513], qsb[0:1, 0, 0:1])
            nc.vector.tensor_mul(gate[:, 1:2], wq_sb[0:1, 0, 128:129], qsb[0:1, 0, 0:1])
            nc.vector.tensor_mul(gate[:, 2:3], wk_sb[0:1, 0, 128:129], qsb[0:1, 0, 0:1])
            nc.scalar.dma_start(
                out=wq_sb[:, :, 128:512],
                in_=wq[:, 128:512].rearrange("(ct P) f -> P ct f", P=128),
            )
            nc.gpsimd.dma_start(
                out=wk_sb[:, :, 128:512],
                in_=wk[:, 128:512].rearrange("(ct P) f -> P ct f", P=128),
            )
            for tc_ in range(1, 4):
                nc.sync.dma_start(
                    out=xt[:, :, ts(tc_, 512)],
                    in_=xa[:, ts(tc_, 512)].rearrange("(ct P) t -> P ct t", P=128),
                )
            nc.gpsimd.dma_start(
                out=wo_sb, in_=wo.rearrange("(np P) f -> P np f", P=128)
            )

            LAG = 2
            all_items = []

            def do_item(it):
                p, qc, nj, jg, first, last = it
                for j in jg:
                    s_exp(p, qc, j)
                all_items.append(it)
                k = len(all_items) - 1
                if k - LAG >= 0:
                    av_item(all_items[k - LAG])

            for r in range(NQC):
                ritems = rounds[r]
                fe, fle = fillers_early[r], fillers_late[r]
                n_it, ne, nl = len(ritems), len(fe), len(fle)
                ei = li = 0
                for ii, it in enumerate(ritems):
                    do_item(it)
                    ewant = ((ii + 1) * ne + n_it - 1) // n_it
                    while ei < min(ewant, ne):
                        fn, args = fe[ei]
                        fn(*args)
                        ei += 1
                    if ii >= LATE_START:
                        lwant = ((ii + 1 - LATE_START) * nl + (n_it - LATE_START) - 1) // max(
                            1, n_it - LATE_START
                        )
                        while li < min(lwant, nl):
                            fn, args = fle[li]
                            fn(*args)
                            li += 1
                while ei < ne:
                    fn, args = fe[ei]
                    fn(*args)
                    ei += 1
                while li < nl:
                    fn, args = fle[li]
                    fn(*args)
                    li += 1

            # drain the AV lag, then the last output-projection round
            for k in range(len(all_items) - LAG, len(all_items)):
                av_item(all_items[k])
            for ft in range(8):
                o_unit(3, ft)

    nc.compile()
    return nc


def kernel(x, W_qkv, b_qkv, W_out, b_out):
    global _last_in_maps
    bf = ml_dtypes.bfloat16
    x = np.asarray(x, dtype=np.float32)
    W_qkv = np.asarray(W_qkv, dtype=np.float32)
    b_qkv = np.asarray(b_qkv, dtype=np.float32)
    W_out = np.asarray(W_out, dtype=np.float32)
    b_out = np.asarray(b_out, dtype=np.float32)
    B = x.shape[0]

    aug = bool(np.any(b_qkv))
    CT = 9 if aug else 8
    if CT not in _CACHE:
        _CACHE[CT] = _build(CT)
    nc = _CACHE[CT]

    # triangle keep-mask for the diagonal 128 block: [p, c] = 1 if c >= p
    tri = (np.arange(128)[None, :] >= np.arange(128)[:, None]).astype(bf)

    in_maps = []
    for core in range(8):
        b, g = core // 2, core % 2
        xa = x[b]
        if aug:
            pad = np.zeros((T, 128), np.float32)
            pad[:, 0] = 1.0
            xa = np.concatenate([xa, pad], axis=1)

        def wslice(col0):
            w = W_qkv[:, col0 + 512 * g : col0 + 512 * g + 512]
            if aug:
                extra = np.zeros((128, 512), np.float32)
                extra[0] = b_qkv[col0 + 512 * g : col0 + 512 * g + 512]
                w = np.concatenate([w, extra], axis=0)
            return np.ascontiguousarray(w.astype(bf))

        in_maps.append(
            {
                "xa": np.ascontiguousarray(xa.T.astype(bf)),
                "wq": wslice(0),
                "wk": wslice(1024),
                "wv": wslice(2048),
                "wo": np.ascontiguousarray(
                    W_out[512 * g : 512 * g + 512, :].astype(bf)
                ),
                "tri": tri,
            }
        )

    _last_in_maps = in_maps
    res = bass_utils.run_bass_kernel_spmd(nc, in_maps, list(range(8))).results
    out = np.empty((B, T, 1024), np.float32)
    for b in range(B):
        acc = res[2 * b]["ot"] + res[2 * b + 1]["ot"]
        out[b] = acc.T + b_out[None, :]
    return out


# revision 24
# speedup vs baseline: 1.0448x; 1.0448x over previous
"""Causal self-attention (B=4, T=2048, D=1024, H=16) on 8 TRN2 NeuronCores.

Sharding: core i = (batch b = i//2, head-group g = i%2). Data parallel on B,
tensor parallel on heads (8 heads per group): qkv_proj columns and out_proj
rows split per head group. Each core computes a partial [D, T] output^T for
its batch; host sums the two group partials per batch, transposes, adds bias.

v2 vs baseline (396.8us): all matmuls in bf16 (1 cycle/row vs the ~2 c/r the
fp32r path measured on HW), x pre-transposed on the host (kills 128 PE
transposes + copies), everything SBUF-resident (no DRAM bounce of Q/K), the
1/sqrt(dh) scale folded into exp's scale immediate, and one unified emission
schedule: projection / output-projection units are interleaved as PE filler
between attention items so the PE never idles while the Scalar engine (exp,
~163us total, dtype-independent 1 elem/cycle/lane) grinds through softmax.

Per-core pipeline:
  proj units: Q^T/K^T[128(2 heads*64d), t] = W_pair.T @ x^T per (pair, 512-t
  chunk); V[t,d] natural per 128-t tile -> vS[k128, ktile, head, 65] with a
  ones column (softmax denominator via the AV matmul).
  attention items (2 k-tiles each): S^T[k,q] = K^T.T @ Q^T row-tiled 2 heads;
  exp(0.125*S) on ACT -> bf16 P^T; triangle mask-mul on diagonal blocks;
  AV: psum[65,512] += V'[k,d+1].T @ P^T accumulated over k-tiles (row 64 =
  denominator). Normalize with reciprocal + gpsimd partition_broadcast.
  o units: out^T[f,t] += Wo_pair[d128,f].T @ OT_pair[d128,t] over pairs.
PSUM: S pool 2x2 banks, AV 2x1, proj/outproj 2x1 = 8 banks.
"""

import numpy as np
import ml_dtypes

import concourse.bacc as bacc
import concourse.tile as tile
import concourse.mybir as mybir
from concourse import bass_utils
from concourse.bass import ts

F32 = mybir.dt.float32
BF16 = mybir.dt.bfloat16
EXP = mybir.ActivationFunctionType.Exp

T = 2048
TT = 16          # t tiles of 128
NP = 4           # head pairs per core
NQC = 4          # q chunks of 512
SCALE = 0.125    # 1/sqrt(64), folded into exp's scale immediate

_CACHE = {}
_last_in_maps = None


def _build(CT):
    """CT = number of 128-row c-tiles in the (possibly bias-augmented) x/W."""
    nc = bacc.Bacc("TRN2", target_bir_lowering=False, debug=False)
    C = CT * 128

    def mm(*args, **kwargs):
        return nc.tensor.matmul(*args, **kwargs)

    xa = nc.dram_tensor("xa", [C, T], BF16, kind="ExternalInput").ap()  # x^T
    wq = nc.dram_tensor("wq", [C, 512], BF16, kind="ExternalInput").ap()
    wk = nc.dram_tensor("wk", [C, 512], BF16, kind="ExternalInput").ap()
    wv = nc.dram_tensor("wv", [C, 512], BF16, kind="ExternalInput").ap()
    wo = nc.dram_tensor("wo", [512, 1024], BF16, kind="ExternalInput").ap()
    tri = nc.dram_tensor("tri", [128, 128], BF16, kind="ExternalInput").ap()
    ot = nc.dram_tensor("ot", [1024, T], F32, kind="ExternalOutput").ap()

    with tile.TileContext(nc) as tc:
        with (
            tc.tile_pool(name="persist", bufs=1) as persist,
            tc.tile_pool(name="ptp", bufs=6) as ptpool,
            tc.tile_pool(name="rsm", bufs=6) as rpool,
            tc.tile_pool(name="rbcp", bufs=2) as rbcpool,
            tc.tile_pool(name="obnc", bufs=3) as opool,
            tc.tile_pool(name="psS", bufs=2, space="PSUM") as psS,
            tc.tile_pool(name="psAv", bufs=2, space="PSUM") as psAv,
            tc.tile_pool(name="psP", bufs=2, space="PSUM") as psP,
        ):
            vS = persist.tile([128, TT, 8, 65], BF16)     # [k128, ktile, head, d+1]
            OT = persist.tile([128, NP, T], BF16)         # [d128(2 heads), pair, t]
            tr = persist.tile([128, 128], BF16)
            wo_sb = persist.tile([128, NP, 1024], BF16)
            qsb = persist.tile([128, NP, T], BF16)        # Q^T per pair
            ksb = persist.tile([128, NP, T], BF16)        # K^T per pair
            xt = persist.tile([128, CT, T], BF16)         # x^T tiles
            wq_sb = persist.tile([128, CT, 512], BF16)
            wk_sb = persist.tile([128, CT, 512], BF16)
            wv_sb = persist.tile([128, CT, 512], BF16)
            nc.vector.memset(vS[:, :, :, 64:65], 1.0)

            # ---- input DMAs. HBM bw (~358GB/s) is shared by every transfer
            # in flight, so the critical first-unit deps (xt chunk 0, the
            # pair-0 columns of wq/wk) are issued alone; all bulk transfers
            # are gated behind a dummy DVE read of their target regions that
            # only runs after the prologue's psum copies (see below).
            nc.sync.dma_start(
                out=xt[:, :, ts(0, 512)],
                in_=xa[:, ts(0, 512)].rearrange("(ct P) t -> P ct t", P=128),
            )
            nc.scalar.dma_start(
                out=wq_sb[:, :, 0:128],
                in_=wq[:, 0:128].rearrange("(ct P) f -> P ct f", P=128),
            )
            nc.gpsimd.dma_start(
                out=wk_sb[:, :, 0:128],
                in_=wk[:, 0:128].rearrange("(ct P) f -> P ct f", P=128),
            )
            nc.gpsimd.dma_start(
                out=wv_sb, in_=wv.rearrange("(ct P) f -> P ct f", P=128)
            )
            nc.gpsimd.dma_start(out=tr, in_=tri)

            # ---------------- PE work units ----------------
            def qk_unit(p, tc_, which):
                w_sb = wq_sb if which == 0 else wk_sb
                dst = qsb if which == 0 else ksb
                ps = psP.tile([128, 512], F32, name="pp", tag="pp")
                for cc in range(CT):
                    mm(
                        ps,
                        lhsT=w_sb[:, cc, ts(p, 128)],
                        rhs=xt[:, cc, ts(tc_, 512)],
                        start=(cc == 0),
                        stop=(cc == CT - 1),
                    )
                nc.vector.tensor_copy(out=dst[:, p, ts(tc_, 512)], in_=ps)

            def v_unit(tt):
                ps = psP.tile([128, 512], F32, name="pp", tag="pp")
                for cc in range(CT):
                    mm(
                        ps,
                        lhsT=xt[:, cc, ts(tt, 128)],
                        rhs=wv_sb[:, cc, :],
                        start=(cc == 0),
                        stop=(cc == CT - 1),
                    )
                nc.vector.tensor_copy(
                    out=vS[:, tt, :, 0:64],
                    in_=ps.rearrange("p (h d) -> p h d", h=8),
                )

            def o_unit(qc, ft):
                ps = psP.tile([128, 512], F32, name="pp", tag="pp")
                for p in range(NP):
                    mm(
                        ps,
                        lhsT=wo_sb[:, p, ts(ft, 128)],
                        rhs=OT[:, p, ts(qc, 512)],
                        start=(p == 0),
                        stop=(p == NP - 1),
                    )
                ob = opool.tile([128, 512], F32)
                nc.vector.tensor_copy(out=ob, in_=ps)
                # alternate queues; sync is idle after the input loads
                eng = nc.sync if ft % 2 == 0 else nc.gpsimd
                eng.dma_start(out=ot[ts(ft, 128), ts(qc, 512)], in_=ob)

            # ---------------- attention ----------------
            pts = {}
            avs = {}

            def s_exp(p, qc, j):
                off = max(0, 128 * j - 512 * qc)
                sg = psS.tile([128, 2, 512], F32, name="sg", tag="sg")
                jo = 512 * (j // 4) + 128 * (j % 4)
                for m in range(2):
                    mm(
                        sg[:, m, off:],
                        lhsT=ksb[64 * m : 64 * m + 64, p, jo : jo + 128],
                        rhs=qsb[64 * m : 64 * m + 64, p, 512 * qc + off : 512 * qc + 512],
                        start=True,
                        stop=True,
                    )
                ptile = ptpool.tile([128, 2, 512], BF16, name="pt", tag="pt")
                nc.scalar.activation(
                    out=ptile[:, :, off:], in_=sg[:, :, off:], func=EXP, scale=SCALE
                )
                if j >= 4 * qc:
                    nc.vector.tensor_mul(
                        ptile[:, :, off : off + 128],
                        ptile[:, :, off : off + 128],
                        tr[:, None, :].to_broadcast([128, 2, 128]),
                    )
                pts[(p, qc, j)] = (ptile, off)

            def av_mm(p, qc, j, nj):
                ptile, off = pts.pop((p, qc, j))
                av = avs[(p, qc)]
                for m in range(2):
                    mm(
                        av[m][:65, off:],
                        lhsT=vS[:, j, 2 * p + m, :],
                        rhs=ptile[:, m, off:],
                        start=(j == 0),
                        stop=(j == nj - 1),
                    )

            def normalize(p, qc):
                # fused: reciprocal straight off the PSUM denominator row,
                # broadcast, then one multiply that reads the PSUM O~ rows
                # and writes normalized bf16 OT (no intermediate copies)
                av = avs.pop((p, qc))
                rrs = []
                for m in range(2):
                    rsb = rpool.tile([1, 512], F32, name="rsb", tag="rsb")
                    nc.vector.tensor_copy(out=rsb, in_=av[m][64:65, :])
                    # unnormalized O~ out of PSUM so the av bank frees fast
                    nc.vector.tensor_copy(
                        out=OT[64 * m : 64 * m + 64, p, ts(qc, 512)],
                        in_=av[m][0:64, :],
                    )
                    rrs.append(rsb)
                for m in range(2):
                    rinv = rpool.tile([1, 512], F32, name="rinv", tag="rinv")
                    nc.vector.reciprocal_approx_fast(out=rinv, in_=rrs[m])
                    rb16 = rpool.tile([1, 512], BF16, name="rb16", tag="rb16")
                    nc.vector.tensor_copy(out=rb16, in_=rinv)
                    rb = rbcpool.tile([128, 512], BF16, name="rb", tag="rb")
                    nc.gpsimd.partition_broadcast(rb, rb16)
                    sl = OT[64 * m : 64 * m + 64, p, ts(qc, 512)]
                    nc.vector.tensor_mul(sl, sl, rb[64 * m : 64 * m + 64, :])

            def av_item(it):
                p, qc, nj, jg, first, last = it
                if first:
                    avs[(p, qc)] = [
                        psAv.tile([128, 512], F32, name="av", tag="av")
                        for _ in range(2)
                    ]
                for j in jg:
                    av_mm(p, qc, j, nj)
                if last:
                    normalize(p, qc)

            # ---------------- emission schedule ----------------
            # items: 2 k-tiles of one (pair, q-chunk); rounds by q-chunk
            rounds = []
            for qc in range(NQC):
                ritems = []
                for p in range(NP):
                    nj = 4 * qc + 4
                    js = list(range(nj))
                    sub = [js[i : i + 2] for i in range(0, nj, 2)]
                    for gi, jg in enumerate(sub):
                        ritems.append((p, qc, nj, jg, gi == 0, gi == len(sub) - 1))
                rounds.append(ritems)

            # per-round PE filler units (produce data for round r+1; drain
            # outproj of round r-1). Order within a round matters: producers
            # must precede their consumers in PE emission order, and o_units
            # of round r-1 must come after normalize(p3, r-1), which is only
            # emitted during item 1 of round r (AV lag) -> late list.
            fillers_early = [
                # round 0: r0 prereqs at the exact slots they are consumed
                # (pair p's qk before item 2p; vS tile j before its AV),
                # then round-1 projections
                [(v_unit, (0,)), (v_unit, (1,)),
                 (qk_unit, (1, 0, 0)), (qk_unit, (1, 0, 1)), (v_unit, (2,)),
                 (qk_unit, (2, 0, 0)), (qk_unit, (2, 0, 1)), (v_unit, (3,)),
                 (qk_unit, (3, 0, 0)), (qk_unit, (3, 0, 1))]
                + [(qk_unit, (p, 1, w)) for p in range(NP) for w in (0, 1)]
                + [(v_unit, (tt,)) for tt in (4, 5, 6, 7)],
                [(qk_unit, (p, 2, w)) for p in range(NP) for w in (0, 1)]
                + [(v_unit, (tt,)) for tt in (8, 9, 10, 11)],
                [(qk_unit, (p, 3, w)) for p in range(NP) for w in (0, 1)]
                + [(v_unit, (tt,)) for tt in (12, 13, 14, 15)],
                [],
            ]
            fillers_late = [
                [],
                [(o_unit, (0, ft)) for ft in range(8)],
                [(o_unit, (1, ft)) for ft in range(8)],
                [(o_unit, (2, ft)) for ft in range(8)],
            ]
            LATE_START = 4  # item index within the round where late fillers may begin

            # prologue: just enough for the first items of pair 0 (vS tiles
            # 0/1 are first fillers -- AV lags 2 items, so they land in time)
            qk_unit(0, 0, 0)
            qk_unit(0, 0, 1)

            # bulk DMAs, staged behind gate ops that read BOTH the bulk
            # target region and a completion signal (real dependency, so the
            # scheduler cannot hoist them): stage A (rest of wq/wk, needed by
            # pair 1 at ~28us) releases when wv -- the last critical DMA --
            # lands (~11us); stage B (xt chunks 1-3, wo) releases on the
            # first qsb copy (~22us), long before anything consumes them.
            gate = rpool.tile([1, 8], BF16, name="gate", tag="gate")
            nc.vector.tensor_mul(
                gate[:, 1:2], wq_sb[0:1, 0, 128:129], wv_sb[0:1, 0, 0:1]
            )
            nc.vector.tensor_mul(
                gate[:, 2:3], wk_sb[0:1, 0, 128:129], wv_sb[0:1, 0, 0:1]
            )
            nc.scalar.dma_start(
                out=wq_sb[:, :, 128:512],
                in_=wq[:, 128:512].rearrange("(ct P) f -> P ct f", P=128),
            )
            nc.gpsimd.dma_start(
                out=wk_sb[:, :, 128:512],
                in_=wk[:, 128:512].rearrange("(ct P) f -> P ct f", P=128),
            )
            for tc_ in range(1, 4):
                nc.vector.tensor_mul(
                    gate[:, 2 + tc_ : 3 + tc_],
                    xt[0:1, 0, 512 * tc_ : 512 * tc_ + 1],
                    qsb[0:1, 0, 0:1],
                )
            nc.vector.tensor_mul(gate[:, 6:7], wo_sb[0:1, 0, 0:1], qsb[0:1, 0, 0:1])
            for tc_ in range(1, 4):
                nc.sync.dma_start(
                    out=xt[:, :, ts(tc_, 512)],
                    in_=xa[:, ts(tc_, 512)].rearrange("(ct P) t -> P ct t", P=128),
                )
            nc.gpsimd.dma_start(
                out=wo_sb, in_=wo.rearrange("(np P) f -> P np f", P=128)
            )

            LAG = 2
            all_items = []

            def do_item(it):
                p, qc, nj, jg, first, last = it
                for j in jg:
                    s_exp(p, qc, j)
                all_items.append(it)
                k = len(all_items) - 1
                if k - LAG >= 0:
                    av_item(all_items[k - LAG])

            for r in range(NQC):
                ritems = rounds[r]
                fe, fle = fillers_early[r], fillers_late[r]
                n_it, ne, nl = len(ritems), len(fe), len(fle)
                ei = li = 0
                for ii, it in enumerate(ritems):
                    do_item(it)
                    ewant = ((ii + 1) * ne + n_it - 1) // n_it
                    while ei < min(ewant, ne):
                        fn, args = fe[ei]
                        fn(*args)
                        ei += 1
                    if ii >= LATE_START:
                        lwant = ((ii + 1 - LATE_START) * nl + (n_it - LATE_START) - 1) // max(
                            1, n_it - LATE_START
                        )
                        while li < min(lwant, nl):
                            fn, args = fle[li]
                            fn(*args)
                            li += 1
                while ei < ne:
                    fn, args = fe[ei]
                    fn(*args)
                    ei += 1
                while li < nl:
                    fn, args = fle[li]
                    fn(*args)
                    li += 1

            # drain the AV lag, then the last output-projection round
            for k in range(len(all_items) - LAG, len(all_items)):
                av_item(all_items[k])
            for ft in range(8):
                o_unit(3, ft)

    nc.compile()
    return nc


def kernel(x, W_qkv, b_qkv, W_out, b_out):
    global _last_in_maps
    bf = ml_dtypes.bfloat16
    x = np.asarray(x, dtype=np.float32)
    W_qkv = np.asarray(W_qkv, dtype=np.float32)
    b_qkv = np.asarray(b_qkv, dtype=np.float32)
    W_out = np.asarray(W_out, dtype=np.float32)
    b_out = np.asarray(b_out, dtype=np.float32)
    B = x.shape[0]

    aug = bool(np.any(b_qkv))
    CT = 9 if aug else 8
    if CT not in _CACHE:
        _CACHE[CT] = _build(CT)
    nc = _CACHE[CT]

    # triangle keep-mask for the diagonal 128 block: [p, c] = 1 if c >= p
    tri = (np.arange(128)[None, :] >= np.arange(128)[:, None]).astype(bf)

    in_maps = []
    for core in range(8):
        b, g = core // 2, core % 2
        xa = x[b]
        if aug:
            pad = np.zeros((T, 128), np.float32)
            pad[:, 0] = 1.0
            xa = np.concatenate([xa, pad], axis=1)

        def wslice(col0):
            w = W_qkv[:, col0 + 512 * g : col0 + 512 * g + 512]
            if aug:
                extra = np.zeros((128, 512), np.float32)
                extra[0] = b_qkv[col0 + 512 * g : col0 + 512 * g + 512]
                w = np.concatenate([w, extra], axis=0)
            return np.ascontiguousarray(w.astype(bf))

        in_maps.append(
            {
                "xa": np.ascontiguousarray(xa.T.astype(bf)),
                "wq": wslice(0),
                "wk": wslice(1024),
                "wv": wslice(2048),
                "wo": np.ascontiguousarray(
                    W_out[512 * g : 512 * g + 512, :].astype(bf)
                ),
                "tri": tri,
            }
        )

    _last_in_maps = in_maps
    res = bass_utils.run_bass_kernel_spmd(nc, in_maps, list(range(8))).results
    out = np.empty((B, T, 1024), np.float32)
    for b in range(B):
        acc = res[2 * b]["ot"] + res[2 * b + 1]["ot"]
        out[b] = acc.T + b_out[None, :]
    return out
